# revision 18
# baseline (speedup 1.0000x reference)
"""Raw-bass Trainium2 kernel for nn_CriticTab, v10.

Algorithm (same as the v7 baseline): mask enumerates all 2^16 binary
states, so out[b] = v[packbits(obs[b])].  Host folds (mask, v) into a
65536-entry table; the device bit-packs obs on DVE and SWDGE-gathers
one f32 per observation.  HW constraint (measured): an indirect DMA
consumes exactly one offset per destination partition, so 512 gathers
need 4 serialized [128,1] calls — that part is structural.

v11 vs v7 (21998 ns measured; v11 ~19.4k max / ~18.9k mean):
  * idx computed per 128-obs column (1 shift + 4 narrow reduces), so
    gather g can issue as soon as column g is packed instead of after
    half the batch.
  * v7's warmup gather removed entirely: the GpSimd library load
    completes ~1.7 us before the first real gather can issue, the
    first-call penalty is only ~120 ns, and dropping it removes one
    SWDGE packet from slow SDMA engine 15's queue.
  * final store issued from the Scalar (ACT) HWDGE queue, and the
    explicit store-completion wait is dropped: the NEFF epilogue's
    per-engine queue DRAIN already fences the DMA, so its completion
    overlaps the (measured ~7 us) end-of-kernel semaphore-file reset
    instead of preceding it.

Architectures measured and rejected on this rig: single-call 512-offset
indirect DMA (HW consumes one offset per dest partition — impossible),
dma_gather (library load ~9 us, ~4.7 us/call), full Tensor-engine
two-level lookup (kernel_pe.py: 23.3k/21.4k — DVE-serialization-bound),
gather+PE hybrid (kernel_hy.py: mean 19.8k but max-core 22-23k from
table-DMA-driven cross-core variance).
"""

import numpy as np

B, D, N = 4096, 16, 65536
N_CORES = 8
BS = B // N_CORES  # 512 observations per core
P = 128            # SBUF partitions
G = BS // P        # 4 observations per partition

_CACHE = {}

LAST_RESULT = None


def build_program():
    if "nc" in _CACHE:
        return _CACHE["nc"]

    import concourse.bacc as bacc
    import concourse.bass as bass
    import concourse.mybir as mybir

    nc = bacc.Bacc("TRN2", debug=False, enable_asserts=False, num_devices=N_CORES)
    obs_d = nc.dram_tensor("obs", [BS, D], mybir.dt.int32, kind="ExternalInput")
    v_d = nc.dram_tensor("v", [N, 1], mybir.dt.float32, kind="ExternalInput")
    out_d = nc.dram_tensor("out", [BS], mybir.dt.float32, kind="ExternalOutput")

    with (
        nc.semaphore("s_obs") as s_obs,
        nc.semaphore("s_iota") as s_iota,
        nc.semaphore("s_idx") as s_idx,
        nc.semaphore("s_g") as s_g,
        nc.semaphore("s_done") as s_done,
        nc.sbuf_tensor("obs_t", [P, G * D], mybir.dt.int32) as obs_t,
        nc.sbuf_tensor("sh_t", [P, G * D], mybir.dt.int32) as sh_t,
        nc.sbuf_tensor("prod_t", [P, G * D], mybir.dt.int32) as prod_t,
        nc.sbuf_tensor("idx_t", [P, G], mybir.dt.int32) as idx_t,
        nc.sbuf_tensor("g_t", [P, G], mybir.dt.float32) as g_t,
    ):
        # Sync: obs load is the first kernel instruction.  Each partition
        # reads 4 contiguous rows (256 B).
        nc.sync.dma_start(
            out=obs_t[:], in_=obs_d[:].rearrange("(p g) d -> p (g d)", p=P)
        ).then_inc(s_obs, 16)

        # GpSimd: shift table (no input deps; runs during the obs DMA).
        # No warmup gather: the SWDGE library load completes ~1.7us before
        # the first real gather can issue, and dropping it removes one
        # packet from slow SDMA engine 15's queue.
        nc.gpsimd.iota(
            sh_t[:], pattern=[[0, G], [1, D]], channel_multiplier=0
        ).then_inc(s_iota, 1)

        # Vector: idx[p, g] = sum_d obs[p, g*D+d] << d, one narrow reduce
        # per column so the first gather can start early.
        nc.vector.wait_ge(s_iota, 1)
        nc.vector.wait_ge(s_obs, 16)
        with nc.allow_low_precision(reason="exact small-int bit packing"):
            nc.vector.tensor_tensor(
                out=prod_t[:],
                in0=obs_t[:],
                in1=sh_t[:],
                op=mybir.AluOpType.logical_shift_left,
            )
            for g in range(G):
                nc.vector.tensor_reduce(
                    out=idx_t[:, g : g + 1],
                    in_=prod_t[:, g * D : (g + 1) * D].rearrange(
                        "p (o d) -> p o d", o=1
                    ),
                    axis=mybir.AxisListType.X,
                    op=mybir.AluOpType.add,
                ).then_inc(s_idx, 1)

        # GpSimd: four [128,1] gathers (one offset per dest partition is
        # a HW constraint), each released as its idx column lands.
        for g in range(G):
            nc.gpsimd.wait_ge(s_idx, g + 1)
            nc.gpsimd.indirect_dma_start(
                out=g_t[:, g : g + 1],
                out_offset=None,
                in_=v_d[:],
                in_offset=bass.IndirectOffsetOnAxis(ap=idx_t[:, g : g + 1], axis=0),
                oob_is_err=False,
            ).then_inc(s_g, 16)

        # Scalar (ACT) HWDGE: store once all gathers have landed.  No
        # completion wait — the NEFF epilogue's queue drain fences it.
        nc.scalar.wait_ge(s_g, 16 * G)
        nc.scalar.dma_start(
            out=out_d[:].rearrange("(p g) -> p g", p=P), in_=g_t[:]
        ).then_inc(s_done, 16)

    nc.compile()
    _CACHE["nc"] = nc
    return nc


def _fold_table(mask: np.ndarray, v: np.ndarray) -> np.ndarray:
    pw = 1 << np.arange(D, dtype=np.int64)
    m01 = (np.asarray(mask).astype(np.int64) + 1) // 2
    keys = (m01 * pw[None, :]).sum(axis=1)
    if np.array_equal(keys, np.arange(N, dtype=np.int64)):
        return v
    table = np.zeros(N, dtype=np.float32)
    np.add.at(table, keys, v)
    return table


def kernel(obs, mask, v):
    global LAST_RESULT
    from concourse.bass_utils import run_bass_kernel_spmd

    obs = np.ascontiguousarray(np.asarray(obs), dtype=np.int32)
    v = np.ascontiguousarray(np.asarray(v), dtype=np.float32)
    table = np.ascontiguousarray(_fold_table(mask, v)).reshape(N, 1)

    nc = build_program()
    in_maps = [
        {"obs": obs[i * BS : (i + 1) * BS], "v": table} for i in range(N_CORES)
    ]
    res = run_bass_kernel_spmd(nc, in_maps, list(range(N_CORES)))
    LAST_RESULT = res
    return np.concatenate(
        [res.results[i]["out"].reshape(BS) for i in range(N_CORES)]
    )
